# revision 13
# baseline (speedup 1.0000x reference)
"""CompressedSparseAttention Trainium2 kernel — 1-core variant, v2.

Sharding: none — a single core processes both batches (loop) and all 8
heads (group loop). Minimizing per-dispatch I/O dominates the measured
time; the device program is additionally restructured for less work:
  - compressed KV pooling runs on x BEFORE projection (linearity), once
    per batch instead of once per head-group
  - window-branch exp + causal masks batched into [128, 512] ops
  - RoPE multiplies emit bf16
  - per-group scratch is double-buffered so consecutive groups overlap

Per-core inputs:
  xbf   [1024, 2048] bf16  x[0].T | x[1].T stacked
  wpk   [512, 3584] bf16   4 group-blocks of 896 cols:
                           wq|wqP|wk|wkP|wv|wkc|wvc for heads (2g, 2g+1),
                           where *P are row-permuted copies (RoPE half-swap
                           folded into the projection weights)
  wob   [512, 512]  bf16   wo.T (rows = head dims)
  misc  [1, 16]     f32    softmax(gate_logits) | exp(sink_logit[0..8))
  outp  [4096, 512] bf16   finished output, batches stacked
cos/sin RoPE tables are Const tensors baked into the NEFF.
"""

import math

import numpy as np

import concourse.bass as bass
import concourse.mybir as mybir
import concourse.tile as tile
from concourse import bacc
from concourse.bass import ds
from concourse.masks import make_identity

B = 2
L = 2048
D = 512
H = 8
HD = 64
RATIO = 8
STRIDE = 4
WINDOW = 128
THETA = 10000.0
LC = (L - RATIO) // STRIDE + 1  # 511
NCORES = 1
NGRP = 4  # head-pair groups
NB = L // 512  # 4 q-blocks of 512
NCH = L // 128  # 16 q-chunks of 128
KD = D // 128  # 4 contraction chunks
NWC = 7  # weight column blocks per group: wq wqP wk wkP wv wkc wvc

F32 = mybir.dt.float32
BF16 = mybir.dt.bfloat16
AF = mybir.ActivationFunctionType
ALU = mybir.AluOpType

_CACHE = {}


def _rope_tables():
    half = HD // 2
    inv_freq = 1.0 / (THETA ** (np.arange(half, dtype=np.float32) / half))
    t = np.arange(L, dtype=np.float32)
    f = t[:, None] * inv_freq[None, :]  # [L, 32]
    cos32 = np.cos(f).T.astype(np.float32)  # [32, L]
    sin32 = np.sin(f).T.astype(np.float32)
    cosT = np.tile(cos32, (4, 1))  # rows: i%32
    sinST = np.concatenate([-sin32, sin32, -sin32, sin32], axis=0)
    return cosT, sinST


def _build_nc():
    nc = bacc.Bacc(
        "TRN2",
        target_bir_lowering=False,
        debug=False,
        num_devices=NCORES,
        name="csa1b",
    )

    xbf_d = nc.dram_tensor("xbf", [B * D, L], BF16, kind="ExternalInput")
    wpk_d = nc.dram_tensor("wpk", [D, 128 * NWC * NGRP], BF16, kind="ExternalInput")
    wob_d = nc.dram_tensor("wob", [D, D], BF16, kind="ExternalInput")
    misc_d = nc.dram_tensor("misc", [1, 16], F32, kind="ExternalInput")
    outp_d = nc.dram_tensor("outp", [B * L, D], BF16, kind="ExternalOutput")

    cos_np, sinS_np = _rope_tables()
    cosT_d = nc.inline_tensor(cos_np, name="cosconst")
    sinST_d = nc.inline_tensor(sinS_np, name="sinconst")

    # 0/1 causal masks (bf16), multiplied into the exp'd scores on DVE.
    # Layout [key p, 4 sub-blocks of 128 q cols]; q = col % 128 within a block.
    import ml_dtypes
    p = np.arange(128)[:, None]
    q = np.tile(np.arange(128)[None, :], (1, 4)).reshape(1, 512)
    q = np.concatenate([np.arange(128)] * 4)[None, :]
    bf = ml_dtypes.bfloat16
    m_wincur = (q >= p).astype(bf)                 # keep q_rel >= k_rel
    m_winprev = (p > q).astype(bf)                 # keep k_rel > q_rel
    # compressed masks use the FULL 512-col q index within the q-block
    qf = np.arange(512)[None, :]
    m_csame = (qf >= 4 * p + 7).astype(bf)         # wc == qb
    m_cprev = (qf + 512 >= 4 * p + 7).astype(bf)   # wc == qb - 1
    mwc_d = nc.inline_tensor(np.ascontiguousarray(m_wincur), name="mwc")
    mwp_d = nc.inline_tensor(np.ascontiguousarray(m_winprev), name="mwp")
    mcs_d = nc.inline_tensor(np.ascontiguousarray(m_csame), name="mcs")
    mcp_d = nc.inline_tensor(np.ascontiguousarray(m_cprev), name="mcp")

    with tile.TileContext(nc) as tc:
        with tc.tile_pool(name="consts", bufs=1) as cp, \
             tc.tile_pool(name="work", bufs=1) as wp, \
             tc.tile_pool(name="ps", bufs=7, space="PSUM") as pp, \
             tc.tile_pool(name="pss", bufs=1, space="PSUM") as pps:

            # ---------------- init: DMA inputs + consts ----------------
            wpk = []
            for c in range(KD):
                t = cp.tile([128, 128 * NWC * NGRP], BF16, tag=f"wpk{c}",
                            name=f"wpk{c}")
                nc.sync.dma_start(out=t, in_=wpk_d[ds(128 * c, 128), :])
                wpk.append(t)

            wob_t = []
            for g in range(NGRP):
                t = cp.tile([128, D], BF16, tag=f"wob{g}", name=f"wob{g}")
                nc.sync.dma_start(out=t, in_=wob_d[ds(128 * g, 128), :])
                wob_t.append(t)

            cosT = cp.tile([128, L], F32, tag="cosT")
            nc.sync.dma_start(out=cosT, in_=cosT_d[:, :])
            sinST = cp.tile([128, L], F32, tag="sinST")
            nc.sync.dma_start(out=sinST, in_=sinST_d[:, :])

            misc = cp.tile([1, 16], F32, tag="misc")
            nc.sync.dma_start(out=misc, in_=misc_d[:, :])
            gateb = cp.tile([128, RATIO], F32, tag="gateb")
            nc.gpsimd.partition_broadcast(gateb, misc[0:1, 0:RATIO])
            expsb = cp.tile([128, H], F32, tag="expsb")
            nc.gpsimd.partition_broadcast(expsb, misc[0:1, RATIO:RATIO + H])

            ident_bf = cp.tile([128, 128], BF16, tag="ident_bf")
            make_identity(nc, ident_bf)
            ident_f = cp.tile([128, 128], F32, tag="ident_f")
            make_identity(nc, ident_f)

            def load_mask(dram, tag):
                t = cp.tile([128, 512], BF16, tag=tag, name=tag)
                nc.sync.dma_start(out=t, in_=dram[:, :])
                return t

            mwc = load_mask(mwc_d, "mwc")
            mwp = load_mask(mwp_d, "mwp")
            mcs = load_mask(mcs_d, "mcs")
            mcp = load_mask(mcp_d, "mcp")

            # persistent per-(group, q-block) attention numerators + recips
            avT = [
                [cp.tile([128, 512], BF16, tag=f"avT{g}_{qb}", name=f"avT{g}_{qb}")
                 for qb in range(NB)]
                for g in range(NGRP)
            ]
            rec = [
                [cp.tile([128, NCH], F32, tag=f"rec{g}{h}", name=f"rec{g}{h}")
                 for h in range(2)]
                for g in range(NGRP)
            ]

            def project(xT, wcol, qb, name):
                ps = pp.tile([128, 512], F32, tag="bank", name=name)
                for c in range(KD):
                    nc.tensor.matmul(
                        ps,
                        wpk[c][:, ds(128 * wcol, 128)],
                        xT[c][:, ds(512 * qb, 512)],
                        start=(c == 0),
                        stop=(c == KD - 1),
                    )
                return ps

            def rope_block(ps1, ps2, outT, qb):
                m1 = wp.tile([128, 512], BF16, tag="m1", bufs=2, name="m1")
                nc.vector.tensor_mul(m1, ps1, cosT[:, ds(512 * qb, 512)])
                m2 = wp.tile([128, 512], BF16, tag="m2", bufs=2, name="m2")
                nc.vector.tensor_mul(m2, ps2, sinST[:, ds(512 * qb, 512)])
                nc.vector.tensor_add(outT[:, ds(512 * qb, 512)], m1, m2)

            def pool(y, out_bf):
                # out_bf[p, w] = sum_r gate[r] * y[p, 4w + r]
                y4 = y.rearrange("p (w r) -> p r w", r=STRIDE)
                acc = [
                    wp.tile([128, LC], F32, tag="poolA", bufs=2, name="poolA"),
                    wp.tile([128, LC], F32, tag="poolB", bufs=2, name="poolB"),
                ]
                nc.vector.tensor_scalar(
                    out=acc[0],
                    in0=y4[:, 0, 0:LC],
                    scalar1=gateb[:, 0:1],
                    scalar2=None,
                    op0=ALU.mult,
                )
                for r in range(1, RATIO):
                    dst = out_bf if r == RATIO - 1 else acc[r % 2]
                    nc.vector.scalar_tensor_tensor(
                        out=dst,
                        in0=y4[:, r % STRIDE, (r // STRIDE):(r // STRIDE) + LC],
                        scalar=gateb[:, ds(r, 1)],
                        in1=acc[(r - 1) % 2],
                        op0=ALU.mult,
                        op1=ALU.add,
                    )

            for b in range(B):
                # x chunks for this batch
                xT = []
                for c in range(KD):
                    xt = wp.tile([128, L], BF16, tag=f"xt{c}", bufs=2,
                                 name=f"xt{c}_{b}")
                    nc.sync.dma_start(
                        out=xt, in_=xbf_d[ds(512 * b + 128 * c, 128), :])
                    xT.append(xt)

                # pooled x (compressed tokens), shared by all groups
                x_cT = []
                for c in range(KD):
                    xc = wp.tile([128, LC], BF16, tag=f"xc{c}", bufs=2,
                                 name=f"xc{c}_{b}")
                    pool(xT[c], xc)
                    x_cT.append(xc)

                for g in range(NGRP):
                    wbase = NWC * g

                    # ---------- P1: projections + RoPE ----------
                    qT = wp.tile([128, L], BF16, tag="qT", bufs=2, name=f"qT{b}{g}")
                    kT = wp.tile([128, L], BF16, tag="kT", bufs=2, name=f"kT{b}{g}")
                    vT_bf = wp.tile([128, L], BF16, tag="vT", bufs=2,
                                    name=f"vT{b}{g}")
                    for qb in range(NB):
                        ps1 = project(xT, wbase + 0, qb, "q1")
                        ps2 = project(xT, wbase + 1, qb, "q2")
                        rope_block(ps1, ps2, qT, qb)
                    for qb in range(NB):
                        ps1 = project(xT, wbase + 2, qb, "k1")
                        ps2 = project(xT, wbase + 3, qb, "k2")
                        rope_block(ps1, ps2, kT, qb)
                    for qb in range(NB):
                        ps = project(xT, wbase + 4, qb, "v")
                        nc.scalar.copy(out=vT_bf[:, ds(512 * qb, 512)], in_=ps)

                    # compressed K/V: project pooled x (511 cols)
                    k_cT = wp.tile([128, LC], BF16, tag="k_cT", bufs=2,
                                   name=f"k_cT{b}{g}")
                    v_cT = wp.tile([128, LC], BF16, tag="v_cT", bufs=2,
                                   name=f"v_cT{b}{g}")
                    for wcol, dst in ((wbase + 5, k_cT), (wbase + 6, v_cT)):
                        ps = pp.tile([128, LC], F32, tag="bank", name="kcvc")
                        for c in range(KD):
                            nc.tensor.matmul(
                                ps,
                                wpk[c][:, ds(128 * wcol, 128)],
                                x_cT[c],
                                start=(c == 0),
                                stop=(c == KD - 1),
                            )
                        nc.scalar.copy(out=dst, in_=ps)

                    # transpose v -> [pos, dim] chunks with ones cols
                    v_aug = []
                    for ch in range(NCH):
                        va = wp.tile([128, 130], BF16, tag=f"v_aug{ch}", bufs=2,
                                     name=f"v_aug{ch}_{b}{g}")
                        nc.gpsimd.memset(va, 1.0)
                        tp = pps.tile([128, 128], BF16, tag="small", name="tr_ps")
                        nc.tensor.transpose(
                            tp, vT_bf[:, ds(128 * ch, 128)], ident_bf)
                        nc.vector.tensor_copy(out=va[:, 0:64], in_=tp[:, 0:64])
                        nc.vector.tensor_copy(out=va[:, 65:129], in_=tp[:, 64:128])
                        v_aug.append(va)

                    vc_aug = []
                    for ch in range(4):
                        wlen = min(128, LC - 128 * ch)  # 128,128,128,127
                        va = wp.tile([128, 130], BF16, tag=f"vc_aug{ch}", bufs=2,
                                     name=f"vc_aug{ch}_{b}{g}")
                        nc.gpsimd.memset(va, 1.0)
                        tp = pps.tile([128, 128], BF16, tag="small", name="trc_ps")
                        nc.tensor.transpose(
                            tp[0:wlen, :], v_cT[:, ds(128 * ch, wlen)], ident_bf
                        )
                        nc.vector.tensor_copy(
                            out=va[0:wlen, 0:64], in_=tp[0:wlen, 0:64])
                        nc.vector.tensor_copy(
                            out=va[0:wlen, 65:129], in_=tp[0:wlen, 64:128])
                        vc_aug.append(va)

                    # ---------- P2: attention ----------
                    # av4 holds, per q-block, the FLIPPED accumulators for the
                    # 4 q-chunks side by side: av4[:, 65j:65j+65] =
                    # [q(128), 64 dims + 1 denominator col] — q on partitions,
                    # so the softmax normalization is a per-partition scale.
                    for qb in range(NB):
                        for h in range(2):
                            hs = 64 * h
                            qs = qT[ds(hs, 64), ds(512 * qb, 512)]
                            avj = [
                                pp.tile([128, 65], F32, tag="bank",
                                        name=f"av_{b}_{g}_{qb}_{h}_{j}")
                                for j in range(4)
                            ]
                            first_av = [True, True, True, True]

                            def av_mm(j, exsl, vsl, stop=False):
                                nc.tensor.matmul(
                                    avj[j],
                                    exsl,
                                    vsl,
                                    start=first_av[j],
                                    stop=stop,
                                    skip_group_check=True,
                                )
                                first_av[j] = False

                            # --- compressed branch ---
                            for wc in range(qb + 1):
                                wlen = min(128, LC - 128 * wc)
                                sc = pp.tile([128, 512], F32, tag="bank",
                                             name="sc_ps")
                                nc.tensor.matmul(
                                    sc[0:wlen, :],
                                    k_cT[ds(hs, 64), ds(128 * wc, wlen)],
                                    qs,
                                    start=True,
                                    stop=True,
                                )
                                ex = wp.tile([128, 512], BF16, tag="exc", bufs=3,
                                             name="exc")
                                nc.scalar.activation(
                                    out=ex[0:wlen, :], in_=sc[0:wlen, :],
                                    func=AF.Exp, scale=0.125,
                                )
                                if wc >= qb - 1:
                                    # keep q_rel >= 4*w_rel + 7 - 512*(qb-wc)
                                    mk = mcs if wc == qb else mcp
                                    nc.vector.tensor_mul(
                                        ex[0:wlen, :], ex[0:wlen, :],
                                        mk[0:wlen, :],
                                    )
                                for j in range(4):
                                    av_mm(
                                        j,
                                        ex[0:wlen, ds(128 * j, 128)],
                                        vc_aug[wc][0:wlen, ds(65 * h, 65)],
                                    )

                            # --- local window branch, batched over sub ---
                            psP = pp.tile([128, 512], F32, tag="bank",
                                          name="winP")
                            psC = pp.tile([128, 512], F32, tag="bank",
                                          name="winC")
                            for sub in range(4):
                                c = 4 * qb + sub
                                qcs = qT[ds(hs, 64), ds(128 * c, 128)]
                                if c > 0:
                                    nc.tensor.matmul(
                                        psP[:, ds(128 * sub, 128)],
                                        kT[ds(hs, 64), ds(128 * (c - 1), 128)],
                                        qcs,
                                        start=True, stop=True,
                                        skip_group_check=True,
                                    )
                                nc.tensor.matmul(
                                    psC[:, ds(128 * sub, 128)],
                                    kT[ds(hs, 64), ds(128 * c, 128)],
                                    qcs,
                                    start=True, stop=True,
                                    skip_group_check=True,
                                )
                            exwP = wp.tile([128, 512], BF16, tag="exwP", bufs=3,
                                           name="exwP")
                            exwC = wp.tile([128, 512], BF16, tag="exwC", bufs=3,
                                           name="exwC")
                            lo = 128 if qb == 0 else 0
                            nsub = 3 if qb == 0 else 4
                            nc.scalar.activation(
                                out=exwP[:, lo:512], in_=psP[:, lo:512],
                                func=AF.Exp, scale=0.125,
                            )
                            nc.scalar.activation(
                                out=exwC, in_=psC, func=AF.Exp, scale=0.125,
                            )
                            # prev chunks: keep k_rel > q_rel
                            nc.vector.tensor_mul(
                                exwP[:, lo:512], exwP[:, lo:512],
                                mwp[:, lo:512],
                            )
                            # current chunks: keep q_rel >= k_rel
                            nc.vector.tensor_mul(exwC, exwC, mwc)
                            for sub in range(4):
                                c = 4 * qb + sub
                                if c > 0:
                                    av_mm(
                                        sub,
                                        exwP[:, ds(128 * sub, 128)],
                                        v_aug[c - 1][:, ds(65 * h, 65)],
                                    )
                                av_mm(
                                    sub,
                                    exwC[:, ds(128 * sub, 128)],
                                    v_aug[c][:, ds(65 * h, 65)],
                                    stop=(sub == 3),
                                )

                            # --- denominator col 64 is already per-partition
                            # in q — no transpose dance; then normalize and
                            # transpose back to [dims, q] for the wo matmul
                            for j in range(4):
                                c = 4 * qb + j
                                dsb = wp.tile([128, 1], F32, tag="dsb", bufs=2,
                                              name="dsb")
                                nc.vector.tensor_scalar(
                                    out=dsb, in0=avj[j][:, 64:65],
                                    scalar1=expsb[:, ds(2 * g + h, 1)],
                                    scalar2=None,
                                    op0=ALU.add,
                                )
                                nc.vector.reciprocal(
                                    out=rec[g][h][:, ds(c, 1)], in_=dsb
                                )
                                avn = wp.tile([128, 64], BF16, tag="avn",
                                              bufs=3, name="avn")
                                nc.scalar.activation(
                                    out=avn, in_=avj[j][:, 0:64],
                                    func=AF.Copy,
                                    scale=rec[g][h][:, ds(c, 1)],
                                )
                                tp = pps.tile([128, 128], BF16, tag="small",
                                              name="avn_ps")
                                nc.tensor.transpose(
                                    tp[0:64, :], avn, ident_bf)
                                nc.vector.tensor_copy(
                                    out=avT[g][qb][ds(hs, 64), ds(128 * j, 128)],
                                    in_=tp[0:64, :],
                                )

                # ------- P3: output projection, all 8 heads accumulated ------
                # avT is already softmax-normalized, so the 8 head
                # contributions sum directly in one PSUM bank per chunk —
                # no DVE chain.
                for qb in range(NB):
                    for sub in range(4):
                        c = 4 * qb + sub
                        wops0 = pp.tile([128, 512], F32, tag="bank", name="wops0")
                        wops1 = pp.tile([128, 512], F32, tag="bank", name="wops1")
                        for g in range(NGRP):
                            nc.tensor.matmul(
                                wops0,
                                avT[g][qb][0:64, ds(128 * sub, 128)],
                                wob_t[g][0:64, :],
                                start=(g == 0),
                                stop=(g == NGRP - 1),
                                skip_group_check=True,
                            )
                        for g in range(NGRP):
                            nc.tensor.matmul(
                                wops1,
                                avT[g][qb][64:128, ds(128 * sub, 128)],
                                wob_t[g][64:128, :],
                                start=(g == 0),
                                stop=(g == NGRP - 1),
                                skip_group_check=True,
                            )
                        osb0 = wp.tile([128, 512], F32, tag="osb0", bufs=2,
                                       name="osb0")
                        nc.scalar.copy(out=osb0, in_=wops0)
                        osb = wp.tile([128, 512], BF16, tag="osb", bufs=3,
                                      name="osb")
                        nc.vector.tensor_add(osb, osb0, wops1)
                        nc.sync.dma_start(
                            out=outp_d[ds(2048 * b + 128 * c, 128), :], in_=osb)

    nc.compile()
    return nc


def _rope_perm():
    """Row permutation realizing the RoPE half-swap within each 64-dim head."""
    p = np.arange(128)
    base = (p // 64) * 64
    lr = p % 64
    return base + (lr + 32) % 64


def _host_prep(inputs):
    """Build the single-core input map from full inputs."""
    import ml_dtypes

    bf16 = ml_dtypes.bfloat16
    x = np.asarray(inputs["x"], dtype=np.float32)
    wq = np.asarray(inputs["wq"], dtype=np.float32)
    wk = np.asarray(inputs["wk"], dtype=np.float32)
    wv = np.asarray(inputs["wv"], dtype=np.float32)
    wo = np.asarray(inputs["wo"], dtype=np.float32)
    wk_c = np.asarray(inputs["wk_c"], dtype=np.float32)
    wv_c = np.asarray(inputs["wv_c"], dtype=np.float32)
    gate_logits = np.asarray(inputs["gate_logits"], dtype=np.float32)
    sink_logit = np.asarray(inputs["sink_logit"], dtype=np.float32)

    g = np.exp(gate_logits - gate_logits.max())
    g = (g / g.sum()).astype(np.float32)

    perm = _rope_perm()

    blocks = []
    for grp in range(NGRP):
        sl = slice(128 * grp, 128 * (grp + 1))
        wq_s = wq[sl, :]
        wk_s = wk[sl, :]
        blocks += [
            wq_s.T,
            wq_s[perm, :].T,
            wk_s.T,
            wk_s[perm, :].T,
            wv[sl, :].T,
            wk_c[sl, :].T,
            wv_c[sl, :].T,
        ]
    wpk = np.concatenate(blocks, axis=1).astype(bf16)  # [D, 3584]

    misc = np.zeros((1, 16), np.float32)
    misc[0, 0:RATIO] = g
    misc[0, RATIO:RATIO + H] = np.exp(sink_logit[:, 0])

    wob = np.ascontiguousarray(wo.T).astype(bf16)

    xstack = np.concatenate([x[0].T, x[1].T], axis=0)  # [2*D, L]
    in_maps = [
        {
            "xbf": np.ascontiguousarray(xstack).astype(bf16),
            "wpk": np.ascontiguousarray(wpk),
            "wob": wob,
            "misc": misc,
        }
    ]
    return in_maps


def kernel(**inputs) -> np.ndarray:
    from concourse.bass_utils import run_bass_kernel_spmd

    if "nc" not in _CACHE:
        _CACHE["nc"] = _build_nc()
    nc = _CACHE["nc"]

    in_maps = _host_prep(inputs)
    res = run_bass_kernel_spmd(nc, in_maps, core_ids=list(range(NCORES)))
    out = res.results[0]["outp"].astype(np.float32).reshape(B, L, D)
    return out


# revision 14
# speedup vs baseline: 2.4094x; 2.4094x over previous
"""CompressedSparseAttention Trainium2 kernel — 1-core variant, v2.

Sharding: none — a single core processes both batches (loop) and all 8
heads (group loop). Minimizing per-dispatch I/O dominates the measured
time; the device program is additionally restructured for less work:
  - compressed KV pooling runs on x BEFORE projection (linearity), once
    per batch instead of once per head-group
  - window-branch exp + causal masks batched into [128, 512] ops
  - RoPE multiplies emit bf16
  - per-group scratch is double-buffered so consecutive groups overlap

Per-core inputs:
  xbf   [1024, 2048] bf16  x[0].T | x[1].T stacked
  wpk   [512, 2560] bf16   4 group-blocks of 640 cols:
                           wq|wqP|wk|wkP|wv|wkc|wvc for heads (2g, 2g+1),
                           where *P are row-permuted copies (RoPE half-swap
                           folded into the projection weights)
  wob   [512, 512]  bf16   wo.T (rows = head dims)
  misc  [1, 16]     f32    softmax(gate_logits) | exp(sink_logit[0..8))
  outp  [4096, 512] bf16   finished output, batches stacked
cos/sin RoPE tables are Const tensors baked into the NEFF.
"""

import math

import numpy as np

import concourse.bass as bass
import concourse.mybir as mybir
import concourse.tile as tile
from concourse import bacc
from concourse.bass import ds
from concourse.masks import make_identity

B = 2
L = 2048
D = 512
H = 8
HD = 64
RATIO = 8
STRIDE = 4
WINDOW = 128
THETA = 10000.0
LC = (L - RATIO) // STRIDE + 1  # 511
NCORES = 1
NGRP = 4  # head-pair groups
NB = L // 512  # 4 q-blocks of 512
NCH = L // 128  # 16 q-chunks of 128
KD = D // 128  # 4 contraction chunks
NWC = 5  # weight column blocks per group: wq wk wv wkc wvc

F32 = mybir.dt.float32
BF16 = mybir.dt.bfloat16
AF = mybir.ActivationFunctionType
ALU = mybir.AluOpType

_CACHE = {}


def _rope_tables():
    half = HD // 2
    inv_freq = 1.0 / (THETA ** (np.arange(half, dtype=np.float32) / half))
    t = np.arange(L, dtype=np.float32)
    f = t[:, None] * inv_freq[None, :]  # [L, 32]
    cos32 = np.cos(f).T.astype(np.float32)  # [32, L]
    sin32 = np.sin(f).T.astype(np.float32)
    cosT = np.tile(cos32, (4, 1))  # rows: i%32
    sinST = np.concatenate([-sin32, sin32, -sin32, sin32], axis=0)
    return cosT, sinST


def _build_nc():
    nc = bacc.Bacc(
        "TRN2",
        target_bir_lowering=False,
        debug=False,
        num_devices=NCORES,
        name="csa1b",
    )

    xbf_d = nc.dram_tensor("xbf", [B * D, L], BF16, kind="ExternalInput")
    wpk_d = nc.dram_tensor("wpk", [D, 128 * NWC * NGRP], BF16, kind="ExternalInput")
    wob_d = nc.dram_tensor("wob", [D, D], BF16, kind="ExternalInput")
    misc_d = nc.dram_tensor("misc", [1, 16], F32, kind="ExternalInput")
    outp_d = nc.dram_tensor("outp", [B * L, D], BF16, kind="ExternalOutput")

    cos_np, sinS_np = _rope_tables()
    cosT_d = nc.inline_tensor(cos_np, name="cosconst")
    sinST_d = nc.inline_tensor(sinS_np, name="sinconst")

    # 0/1 causal masks (bf16), multiplied into the exp'd scores on DVE.
    # Layout [key p, 4 sub-blocks of 128 q cols]; q = col % 128 within a block.
    import ml_dtypes
    p = np.arange(128)[:, None]
    q = np.tile(np.arange(128)[None, :], (1, 4)).reshape(1, 512)
    q = np.concatenate([np.arange(128)] * 4)[None, :]
    bf = ml_dtypes.bfloat16
    m_wincur = (q >= p).astype(bf)                 # keep q_rel >= k_rel
    m_winprev = (p > q).astype(bf)                 # keep k_rel > q_rel
    # compressed masks use the FULL 512-col q index within the q-block
    qf = np.arange(512)[None, :]
    m_csame = (qf >= 4 * p + 7).astype(bf)         # wc == qb
    m_cprev = (qf + 512 >= 4 * p + 7).astype(bf)   # wc == qb - 1
    perm = _rope_perm()
    pmat_np = np.zeros((128, 128), dtype=bf)
    pmat_np[np.arange(128), perm] = 1
    pmat_d = nc.inline_tensor(np.ascontiguousarray(pmat_np), name="pmat")
    mwc_d = nc.inline_tensor(np.ascontiguousarray(m_wincur), name="mwc")
    mwp_d = nc.inline_tensor(np.ascontiguousarray(m_winprev), name="mwp")
    mcs_d = nc.inline_tensor(np.ascontiguousarray(m_csame), name="mcs")
    mcp_d = nc.inline_tensor(np.ascontiguousarray(m_cprev), name="mcp")

    with tile.TileContext(nc) as tc:
        with tc.tile_pool(name="consts", bufs=1) as cp, \
             tc.tile_pool(name="work", bufs=1) as wp, \
             tc.tile_pool(name="ps", bufs=7, space="PSUM") as pp, \
             tc.tile_pool(name="pss", bufs=1, space="PSUM") as pps:

            # ---------------- init: DMA inputs + consts ----------------
            wpk = []
            for c in range(KD):
                t = cp.tile([128, 128 * NWC * NGRP], BF16, tag=f"wpk{c}",
                            name=f"wpk{c}")
                nc.sync.dma_start(out=t, in_=wpk_d[ds(128 * c, 128), :])
                wpk.append(t)

            wob_t = []
            for g in range(NGRP):
                t = cp.tile([128, D], BF16, tag=f"wob{g}", name=f"wob{g}")
                nc.sync.dma_start(out=t, in_=wob_d[ds(128 * g, 128), :])
                wob_t.append(t)

            cosT = cp.tile([128, L], F32, tag="cosT")
            nc.sync.dma_start(out=cosT, in_=cosT_d[:, :])
            sinST = cp.tile([128, L], F32, tag="sinST")
            nc.sync.dma_start(out=sinST, in_=sinST_d[:, :])

            misc = cp.tile([1, 16], F32, tag="misc")
            nc.sync.dma_start(out=misc, in_=misc_d[:, :])
            gateb = cp.tile([128, RATIO], F32, tag="gateb")
            nc.gpsimd.partition_broadcast(gateb, misc[0:1, 0:RATIO])
            expsb = cp.tile([128, H], F32, tag="expsb")
            nc.gpsimd.partition_broadcast(expsb, misc[0:1, RATIO:RATIO + H])

            ident_bf = cp.tile([128, 128], BF16, tag="ident_bf")
            make_identity(nc, ident_bf)
            ident_f = cp.tile([128, 128], F32, tag="ident_f")
            make_identity(nc, ident_f)

            def load_mask(dram, tag):
                t = cp.tile([128, 512], BF16, tag=tag, name=tag)
                nc.sync.dma_start(out=t, in_=dram[:, :])
                return t

            pmat = cp.tile([128, 128], BF16, tag="pmat", name="pmat")
            nc.sync.dma_start(out=pmat, in_=pmat_d[:, :])
            mwc = load_mask(mwc_d, "mwc")
            mwp = load_mask(mwp_d, "mwp")
            mcs = load_mask(mcs_d, "mcs")
            mcp = load_mask(mcp_d, "mcp")

            # persistent per-(group, q-block) attention numerators + recips
            avT = [
                [cp.tile([128, 512], BF16, tag=f"avT{g}_{qb}", name=f"avT{g}_{qb}")
                 for qb in range(NB)]
                for g in range(NGRP)
            ]
            rec = [
                [cp.tile([128, NCH], F32, tag=f"rec{g}{h}", name=f"rec{g}{h}")
                 for h in range(2)]
                for g in range(NGRP)
            ]

            def project(xT, wcol, qb, name):
                ps = pp.tile([128, 512], F32, tag="bank", name=name)
                for c in range(KD):
                    nc.tensor.matmul(
                        ps,
                        wpk[c][:, ds(128 * wcol, 128)],
                        xT[c][:, ds(512 * qb, 512)],
                        start=(c == 0),
                        stop=(c == KD - 1),
                    )
                return ps

            def rope_block(ps1, ps2, outT, qb):
                m1 = wp.tile([128, 512], BF16, tag="m1", bufs=2, name="m1")
                nc.vector.tensor_mul(m1, ps1, cosT[:, ds(512 * qb, 512)])
                m2 = wp.tile([128, 512], BF16, tag="m2", bufs=2, name="m2")
                nc.vector.tensor_mul(m2, ps2, sinST[:, ds(512 * qb, 512)])
                nc.vector.tensor_add(outT[:, ds(512 * qb, 512)], m1, m2)

            def pool(y, out_bf):
                # out_bf[p, w] = sum_r gate[r] * y[p, 4w + r]
                y4 = y.rearrange("p (w r) -> p r w", r=STRIDE)
                acc = [
                    wp.tile([128, LC], F32, tag="poolA", bufs=2, name="poolA"),
                    wp.tile([128, LC], F32, tag="poolB", bufs=2, name="poolB"),
                ]
                nc.vector.tensor_scalar(
                    out=acc[0],
                    in0=y4[:, 0, 0:LC],
                    scalar1=gateb[:, 0:1],
                    scalar2=None,
                    op0=ALU.mult,
                )
                for r in range(1, RATIO):
                    dst = out_bf if r == RATIO - 1 else acc[r % 2]
                    nc.vector.scalar_tensor_tensor(
                        out=dst,
                        in0=y4[:, r % STRIDE, (r // STRIDE):(r // STRIDE) + LC],
                        scalar=gateb[:, ds(r, 1)],
                        in1=acc[(r - 1) % 2],
                        op0=ALU.mult,
                        op1=ALU.add,
                    )

            for b in range(B):
                # x chunks for this batch
                xT = []
                for c in range(KD):
                    xt = wp.tile([128, L], BF16, tag=f"xt{c}", bufs=2,
                                 name=f"xt{c}_{b}")
                    nc.sync.dma_start(
                        out=xt, in_=xbf_d[ds(512 * b + 128 * c, 128), :])
                    xT.append(xt)

                # pooled x (compressed tokens), shared by all groups
                x_cT = []
                for c in range(KD):
                    xc = wp.tile([128, LC], BF16, tag=f"xc{c}", bufs=2,
                                 name=f"xc{c}_{b}")
                    pool(xT[c], xc)
                    x_cT.append(xc)

                for g in range(NGRP):
                    wbase = NWC * g

                    # ---------- P1: projections + RoPE ----------
                    qT = wp.tile([128, L], BF16, tag="qT", bufs=2, name=f"qT{b}{g}")
                    kT = wp.tile([128, L], BF16, tag="kT", bufs=2, name=f"kT{b}{g}")
                    vT_bf = wp.tile([128, L], BF16, tag="vT", bufs=2,
                                    name=f"vT{b}{g}")
                    def rope_pair(wcol, outT, qb, nm):
                        ps1 = project(xT, wbase + wcol, qb, nm)
                        raw = wp.tile([128, 512], BF16, tag="qraw", bufs=2,
                                      name="qraw")
                        nc.scalar.copy(out=raw, in_=ps1)
                        ps2 = pp.tile([128, 512], F32, tag="bank", name=nm + "p")
                        nc.tensor.matmul(ps2, pmat, raw, start=True, stop=True)
                        rope_block(ps1, ps2, outT, qb)

                    for qb in range(NB):
                        rope_pair(0, qT, qb, "q1")
                    for qb in range(NB):
                        rope_pair(1, kT, qb, "k1")
                    for qb in range(NB):
                        ps = project(xT, wbase + 2, qb, "v")
                        nc.scalar.copy(out=vT_bf[:, ds(512 * qb, 512)], in_=ps)

                    # compressed K/V: project pooled x (511 cols)
                    k_cT = wp.tile([128, LC], BF16, tag="k_cT", bufs=2,
                                   name=f"k_cT{b}{g}")
                    v_cT = wp.tile([128, LC], BF16, tag="v_cT", bufs=2,
                                   name=f"v_cT{b}{g}")
                    for wcol, dst in ((wbase + 3, k_cT), (wbase + 4, v_cT)):
                        ps = pp.tile([128, LC], F32, tag="bank", name="kcvc")
                        for c in range(KD):
                            nc.tensor.matmul(
                                ps,
                                wpk[c][:, ds(128 * wcol, 128)],
                                x_cT[c],
                                start=(c == 0),
                                stop=(c == KD - 1),
                            )
                        nc.scalar.copy(out=dst, in_=ps)

                    # transpose v -> [pos, dim] chunks with ones cols
                    v_aug = []
                    for ch in range(NCH):
                        va = wp.tile([128, 130], BF16, tag=f"v_aug{ch}", bufs=2,
                                     name=f"v_aug{ch}_{b}{g}")
                        nc.gpsimd.memset(va, 1.0)
                        tp = pps.tile([128, 128], BF16, tag="small", name="tr_ps")
                        nc.tensor.transpose(
                            tp, vT_bf[:, ds(128 * ch, 128)], ident_bf)
                        nc.vector.tensor_copy(out=va[:, 0:64], in_=tp[:, 0:64])
                        nc.vector.tensor_copy(out=va[:, 65:129], in_=tp[:, 64:128])
                        v_aug.append(va)

                    vc_aug = []
                    for ch in range(4):
                        wlen = min(128, LC - 128 * ch)  # 128,128,128,127
                        va = wp.tile([128, 130], BF16, tag=f"vc_aug{ch}", bufs=2,
                                     name=f"vc_aug{ch}_{b}{g}")
                        nc.gpsimd.memset(va, 1.0)
                        tp = pps.tile([128, 128], BF16, tag="small", name="trc_ps")
                        nc.tensor.transpose(
                            tp[0:wlen, :], v_cT[:, ds(128 * ch, wlen)], ident_bf
                        )
                        nc.vector.tensor_copy(
                            out=va[0:wlen, 0:64], in_=tp[0:wlen, 0:64])
                        nc.vector.tensor_copy(
                            out=va[0:wlen, 65:129], in_=tp[0:wlen, 64:128])
                        vc_aug.append(va)

                    # ---------- P2: attention ----------
                    # av4 holds, per q-block, the FLIPPED accumulators for the
                    # 4 q-chunks side by side: av4[:, 65j:65j+65] =
                    # [q(128), 64 dims + 1 denominator col] — q on partitions,
                    # so the softmax normalization is a per-partition scale.
                    for qb in range(NB):
                        for h in range(2):
                            hs = 64 * h
                            qs = qT[ds(hs, 64), ds(512 * qb, 512)]
                            avj = [
                                pp.tile([128, 65], F32, tag="bank",
                                        name=f"av_{b}_{g}_{qb}_{h}_{j}")
                                for j in range(4)
                            ]
                            first_av = [True, True, True, True]

                            def av_mm(j, exsl, vsl, stop=False):
                                nc.tensor.matmul(
                                    avj[j],
                                    exsl,
                                    vsl,
                                    start=first_av[j],
                                    stop=stop,
                                    skip_group_check=True,
                                )
                                first_av[j] = False

                            # --- compressed branch ---
                            for wc in range(qb + 1):
                                wlen = min(128, LC - 128 * wc)
                                sc = pp.tile([128, 512], F32, tag="bank",
                                             name="sc_ps")
                                nc.tensor.matmul(
                                    sc[0:wlen, :],
                                    k_cT[ds(hs, 64), ds(128 * wc, wlen)],
                                    qs,
                                    start=True,
                                    stop=True,
                                )
                                ex = wp.tile([128, 512], BF16, tag="exc", bufs=3,
                                             name="exc")
                                nc.scalar.activation(
                                    out=ex[0:wlen, :], in_=sc[0:wlen, :],
                                    func=AF.Exp, scale=0.125,
                                )
                                if wc >= qb - 1:
                                    # keep q_rel >= 4*w_rel + 7 - 512*(qb-wc)
                                    mk = mcs if wc == qb else mcp
                                    nc.vector.tensor_mul(
                                        ex[0:wlen, :], ex[0:wlen, :],
                                        mk[0:wlen, :],
                                    )
                                for j in range(4):
                                    av_mm(
                                        j,
                                        ex[0:wlen, ds(128 * j, 128)],
                                        vc_aug[wc][0:wlen, ds(65 * h, 65)],
                                    )

                            # --- local window branch, batched over sub ---
                            psP = pp.tile([128, 512], F32, tag="bank",
                                          name="winP")
                            psC = pp.tile([128, 512], F32, tag="bank",
                                          name="winC")
                            for sub in range(4):
                                c = 4 * qb + sub
                                qcs = qT[ds(hs, 64), ds(128 * c, 128)]
                                if c > 0:
                                    nc.tensor.matmul(
                                        psP[:, ds(128 * sub, 128)],
                                        kT[ds(hs, 64), ds(128 * (c - 1), 128)],
                                        qcs,
                                        start=True, stop=True,
                                        skip_group_check=True,
                                    )
                                nc.tensor.matmul(
                                    psC[:, ds(128 * sub, 128)],
                                    kT[ds(hs, 64), ds(128 * c, 128)],
                                    qcs,
                                    start=True, stop=True,
                                    skip_group_check=True,
                                )
                            exwP = wp.tile([128, 512], BF16, tag="exwP", bufs=3,
                                           name="exwP")
                            exwC = wp.tile([128, 512], BF16, tag="exwC", bufs=3,
                                           name="exwC")
                            lo = 128 if qb == 0 else 0
                            nsub = 3 if qb == 0 else 4
                            nc.scalar.activation(
                                out=exwP[:, lo:512], in_=psP[:, lo:512],
                                func=AF.Exp, scale=0.125,
                            )
                            nc.scalar.activation(
                                out=exwC, in_=psC, func=AF.Exp, scale=0.125,
                            )
                            # prev chunks: keep k_rel > q_rel
                            nc.vector.tensor_mul(
                                exwP[:, lo:512], exwP[:, lo:512],
                                mwp[:, lo:512],
                            )
                            # current chunks: keep q_rel >= k_rel
                            nc.vector.tensor_mul(exwC, exwC, mwc)
                            for sub in range(4):
                                c = 4 * qb + sub
                                if c > 0:
                                    av_mm(
                                        sub,
                                        exwP[:, ds(128 * sub, 128)],
                                        v_aug[c - 1][:, ds(65 * h, 65)],
                                    )
                                av_mm(
                                    sub,
                                    exwC[:, ds(128 * sub, 128)],
                                    v_aug[c][:, ds(65 * h, 65)],
                                    stop=(sub == 3),
                                )

                            # --- denominator col 64 is already per-partition
                            # in q — no transpose dance; then normalize and
                            # transpose back to [dims, q] for the wo matmul
                            for j in range(4):
                                c = 4 * qb + j
                                dsb = wp.tile([128, 1], F32, tag="dsb", bufs=2,
                                              name="dsb")
                                nc.vector.tensor_scalar(
                                    out=dsb, in0=avj[j][:, 64:65],
                                    scalar1=expsb[:, ds(2 * g + h, 1)],
                                    scalar2=None,
                                    op0=ALU.add,
                                )
                                nc.vector.reciprocal(
                                    out=rec[g][h][:, ds(c, 1)], in_=dsb
                                )
                                avn = wp.tile([128, 64], BF16, tag="avn",
                                              bufs=3, name="avn")
                                nc.scalar.activation(
                                    out=avn, in_=avj[j][:, 0:64],
                                    func=AF.Copy,
                                    scale=rec[g][h][:, ds(c, 1)],
                                )
                                tp = pps.tile([128, 128], BF16, tag="small",
                                              name="avn_ps")
                                nc.tensor.transpose(
                                    tp[0:64, :], avn, ident_bf)
                                nc.vector.tensor_copy(
                                    out=avT[g][qb][ds(hs, 64), ds(128 * j, 128)],
                                    in_=tp[0:64, :],
                                )

                # ------- P3: output projection, all 8 heads accumulated ------
                # avT is already softmax-normalized, so the 8 head
                # contributions sum directly in one PSUM bank per chunk —
                # no DVE chain.
                for qb in range(NB):
                    for sub in range(4):
                        c = 4 * qb + sub
                        wops0 = pp.tile([128, 512], F32, tag="bank", name="wops0")
                        wops1 = pp.tile([128, 512], F32, tag="bank", name="wops1")
                        for g in range(NGRP):
                            nc.tensor.matmul(
                                wops0,
                                avT[g][qb][0:64, ds(128 * sub, 128)],
                                wob_t[g][0:64, :],
                                start=(g == 0),
                                stop=(g == NGRP - 1),
                                skip_group_check=True,
                            )
                        for g in range(NGRP):
                            nc.tensor.matmul(
                                wops1,
                                avT[g][qb][64:128, ds(128 * sub, 128)],
                                wob_t[g][64:128, :],
                                start=(g == 0),
                                stop=(g == NGRP - 1),
                                skip_group_check=True,
                            )
                        osb0 = wp.tile([128, 512], F32, tag="osb0", bufs=2,
                                       name="osb0")
                        nc.scalar.copy(out=osb0, in_=wops0)
                        osb = wp.tile([128, 512], BF16, tag="osb", bufs=3,
                                      name="osb")
                        nc.vector.tensor_add(osb, osb0, wops1)
                        nc.sync.dma_start(
                            out=outp_d[ds(2048 * b + 128 * c, 128), :], in_=osb)

    nc.compile()
    return nc


def _rope_perm():
    """Row permutation realizing the RoPE half-swap within each 64-dim head."""
    p = np.arange(128)
    base = (p // 64) * 64
    lr = p % 64
    return base + (lr + 32) % 64


def _host_prep(inputs):
    """Build the single-core input map from full inputs."""
    import ml_dtypes

    bf16 = ml_dtypes.bfloat16
    x = np.asarray(inputs["x"], dtype=np.float32)
    wq = np.asarray(inputs["wq"], dtype=np.float32)
    wk = np.asarray(inputs["wk"], dtype=np.float32)
    wv = np.asarray(inputs["wv"], dtype=np.float32)
    wo = np.asarray(inputs["wo"], dtype=np.float32)
    wk_c = np.asarray(inputs["wk_c"], dtype=np.float32)
    wv_c = np.asarray(inputs["wv_c"], dtype=np.float32)
    gate_logits = np.asarray(inputs["gate_logits"], dtype=np.float32)
    sink_logit = np.asarray(inputs["sink_logit"], dtype=np.float32)

    g = np.exp(gate_logits - gate_logits.max())
    g = (g / g.sum()).astype(np.float32)

    perm = _rope_perm()

    blocks = []
    for grp in range(NGRP):
        sl = slice(128 * grp, 128 * (grp + 1))
        wq_s = wq[sl, :]
        wk_s = wk[sl, :]
        blocks += [
            wq_s.T,
            wk_s.T,
            wv[sl, :].T,
            wk_c[sl, :].T,
            wv_c[sl, :].T,
        ]
    wpk = np.concatenate(blocks, axis=1).astype(bf16)  # [D, 2560]

    misc = np.zeros((1, 16), np.float32)
    misc[0, 0:RATIO] = g
    misc[0, RATIO:RATIO + H] = np.exp(sink_logit[:, 0])

    wob = np.ascontiguousarray(wo.T).astype(bf16)

    xstack = np.concatenate([x[0].T, x[1].T], axis=0)  # [2*D, L]
    in_maps = [
        {
            "xbf": np.ascontiguousarray(xstack).astype(bf16),
            "wpk": np.ascontiguousarray(wpk),
            "wob": wob,
            "misc": misc,
        }
    ]
    return in_maps


def kernel(**inputs) -> np.ndarray:
    from concourse.bass_utils import run_bass_kernel_spmd

    if "nc" not in _CACHE:
        _CACHE["nc"] = _build_nc()
    nc = _CACHE["nc"]

    in_maps = _host_prep(inputs)
    res = run_bass_kernel_spmd(nc, in_maps, core_ids=list(range(NCORES)))
    out = res.results[0]["outp"].astype(np.float32).reshape(B, L, D)
    return out
